# revision 1
# baseline (speedup 1.0000x reference)
"""MoE (top-2 of 8 experts, swiglu MLP) Trainium2 kernel.

Strategy (expert parallelism, per the sharding hint):
  - Host computes the gate in float64 (scores = x @ gate_w.T, top-2,
    softmax over the selected pair) and dispatches each token to its two
    experts: this is the "all-to-all by top-k expert index" shard step.
  - Core e receives expert e's weights plus the gathered tokens routed to
    it (transposed, [D, C] with C a common padded capacity, fp16) and
    computes  y = (silu(x @ w1) * (x @ w3)) @ w2 * coef[token]
    on device: fp16 matmul operands, fp32 PSUM accumulation, fp32
    epilogue.  Stage 1 streams w1/w3 (packed per h-tile) and produces the
    swiglu activation g with h on partitions; stage 2 keeps w2 resident
    in SBUF and contracts over h, scaling by the gate coefficient during
    the PSUM->SBUF copy.
  - Host scatter-adds each expert's [C, D] result back into the output.

Shapes: B=4, S=2048, D=1024, H=2816, E=8, K=2.
"""

import numpy as np

B, S, D, HID, E, TOPK = 4, 2048, 1024, 2816, 8, 2
P = 128
DSUB = D // P
HT = HID // P
NCORES = 8
CW = 1024          # tokens per chunk

_nc_cache: dict[int, object] = {}


def _chunks(C):
    # Chunks of up to CW tokens, split so every 512-wide matmul subtile
    # keeps a moving dim >= 256 (N=128 streams at ~0.66 ns/col vs ~0.42
    # for N>=256): a remainder in (CW, CW+384] becomes [rem-384, 384]
    # instead of [CW, small-tail].
    out, c0 = [], 0
    while c0 < C:
        rem = C - c0
        if rem <= CW:
            w = rem
        elif rem <= CW + 384:
            w = rem - 384
        else:
            w = CW
        out.append((c0, w))
        c0 += w
    return out


def _build(C):
    import concourse.tile as tile
    from concourse import bacc, mybir

    F32, F16 = mybir.dt.float32, mybir.dt.float16
    SILU = mybir.ActivationFunctionType.Silu
    MULT = mybir.AluOpType.mult

    nc = bacc.Bacc("TRN2", target_bir_lowering=False, debug=False,
                   num_devices=NCORES)
    xT = nc.dram_tensor("xT", [D, C], F16, kind="ExternalInput")
    w13 = nc.dram_tensor("w13", [D, HT, 2 * P], F16, kind="ExternalInput")
    w2 = nc.dram_tensor("w2", [HID, D], F16, kind="ExternalInput")
    coef = nc.dram_tensor("coef", [P, C // P], F32, kind="ExternalInput")
    y = nc.dram_tensor("y", [C, D], F32, kind="ExternalOutput")

    xT_r = xT.ap().rearrange("(do dp) c -> dp do c", dp=P)
    w13_r = w13.ap().rearrange("(do dp) ht z -> dp do ht z", dp=P)
    w2_r = w2.ap().rearrange("(ho hp) d -> hp ho d", hp=P)

    with tile.TileContext(nc) as tc:
        with tc.tile_pool(name="wts", bufs=1) as wts, \
             tc.tile_pool(name="xpool", bufs=2) as xpool, \
             tc.tile_pool(name="wpool", bufs=10) as wpool, \
             tc.tile_pool(name="gpool", bufs=1) as gpool, \
             tc.tile_pool(name="ypool", bufs=2) as ypool, \
             tc.tile_pool(name="tpool", bufs=2) as tpool, \
             tc.tile_pool(name="psum", bufs=1, space="PSUM") as psum, \
             tc.tile_pool(name="psum2", bufs=2, space="PSUM") as psum2:
            w2_sb = wts.tile([P, HT, D], F16, tag="w2")
            coef_sb = wts.tile([P, C // P], F32, tag="coef")

            maxW = max(w for _, w in _chunks(C))
            for ic, (c0, W) in enumerate(_chunks(C)):
                # moving-dim subtiles, all >= 256 cols where possible
                subs, rem = [], W
                while rem:
                    s = 384 if (rem % 512 == 128 and rem > 128) \
                        else min(512, rem)
                    subs.append(s)
                    rem -= s
                nsub = len(subs)
                xc = xpool.tile([P, DSUB, maxW], F16, tag="x")
                for ds_ in range(DSUB):
                    nc.scalar.dma_start(xc[:, ds_, :W],
                                        xT_r[:, ds_, c0:c0 + W])
                g = gpool.tile([P, HT, maxW], F16, tag="g")
                for ht in range(HT):
                    wc = wpool.tile([P, DSUB, 2 * P], F16, tag="w13")
                    if ic == 0:
                        # cold-ring window: split per projection so the ph1
                        # matmul group only waits on the w1 half
                        nc.sync.dma_start(wc[:, :, :P], w13_r[:, :, ht, :P])
                        nc.sync.dma_start(wc[:, :, P:], w13_r[:, :, ht, P:])
                    else:
                        nc.sync.dma_start(wc[:], w13_r[:, :, ht, :])
                    if ic == 0:
                        nc.scalar.dma_start(w2_sb[:, ht, :], w2_r[:, ht, :])
                        if ht == 0:
                            nc.scalar.dma_start(coef_sb[:], coef.ap())
                    phs = []
                    for half, col in ((0, 0), (1, P)):
                        lo = 0
                        for sub, w_ in enumerate(subs):
                            lo = sum(subs[:sub])
                            ph = psum.tile([P, 512], F32,
                                           tag=f"ph{half}{sub}")
                            for ds_ in range(DSUB):
                                nc.tensor.matmul(
                                    ph[:, :w_], wc[:, ds_, col:col + P],
                                    xc[:, ds_, lo:lo + w_],
                                    start=(ds_ == 0), stop=(ds_ == DSUB - 1))
                            phs.append((ph, lo, w_))
                    for (ph1, lo, w_), (ph3, _, _) in zip(phs[:nsub],
                                                          phs[nsub:]):
                        tmp = tpool.tile([P, 512], F32, tag="tmp")
                        nc.scalar.activation(tmp[:, :w_], ph1[:, :w_], SILU)
                        nc.vector.tensor_tensor(g[:, ht, lo:lo + w_],
                                                tmp[:, :w_], ph3[:, :w_],
                                                MULT)

                for cs in range(W // P):
                    ci = c0 // P + cs
                    ysb = ypool.tile([P, D], F32, tag="y")
                    for dt_ in range(D // 512):
                        py_ = psum2.tile([P, 512], F32, tag="py")
                        for ht in range(HT):
                            nc.tensor.matmul(
                                py_[:], g[:, ht, cs * P:(cs + 1) * P],
                                w2_sb[:, ht, dt_ * 512:(dt_ + 1) * 512],
                                start=(ht == 0), stop=(ht == HT - 1))
                        nc.vector.tensor_scalar_mul(
                            ysb[:, dt_ * 512:(dt_ + 1) * 512], py_[:],
                            coef_sb[:, ci, None])
                    nc.scalar.dma_start(
                        y.ap()[c0 + cs * P: c0 + (cs + 1) * P, :], ysb[:])
    nc.compile()
    return nc


def _get_nc(C):
    if C not in _nc_cache:
        _nc_cache[C] = _build(C)
    return _nc_cache[C]


def _route(xt, gate_w):
    T = xt.shape[0]
    scores = xt.astype(np.float64) @ gate_w.astype(np.float64).T
    ar = np.arange(T)
    i1 = np.argmax(scores, 1)
    s1 = scores[ar, i1]
    scores[ar, i1] = -np.inf
    i2 = np.argmax(scores, 1)
    s2 = scores[ar, i2]
    e2 = np.exp(s2 - s1)
    denom = 1.0 + e2
    return i1, i2, 1.0 / denom, e2 / denom


def _ensure_axon_hooks():
    """bass_utils imports antenv.axon_hooks when tracing is requested
    (e.g. BASS_TRACE=1); some images lack that module. Register a shim
    backed by the boot ctypes NTFF hook so tracing works instead of
    crashing."""
    try:
        import antenv.axon_hooks  # noqa: F401
        return
    except ImportError:
        pass
    import sys
    import types
    hook = None
    try:
        from trn_agent_boot.trn_boot import _ntff_profile_via_ctypes
        hook = _ntff_profile_via_ctypes("/opt/axon/libaxon_pjrt.so")
    except Exception:
        hook = None
    try:
        import antenv
    except ImportError:
        return
    mod = types.ModuleType("antenv.axon_hooks")
    mod.get_axon_ntff_profile_hook = lambda: hook
    mod.set_axon_ntff_profile_hook = lambda h: None
    sys.modules["antenv.axon_hooks"] = mod
    antenv.axon_hooks = mod


def kernel(x, gate_w, w1, w3, w2):
    _ensure_axon_hooks()
    from concourse.bass_utils import run_bass_kernel_spmd

    x = np.asarray(x, dtype=np.float32)
    gate_w = np.asarray(gate_w, dtype=np.float32)
    w1 = np.asarray(w1, dtype=np.float32)
    w3 = np.asarray(w3, dtype=np.float32)
    w2 = np.asarray(w2, dtype=np.float32)

    b, s, d = x.shape
    T = b * s
    xt = x.reshape(T, d)
    i1, i2, wa, wb = _route(xt, gate_w)

    idxs, coefs = [], []
    for e in range(E):
        m1 = i1 == e
        m2 = i2 == e
        cf = np.where(m1, wa, 0.0) + np.where(m2, wb, 0.0)
        idx = np.nonzero(m1 | m2)[0]
        idxs.append(idx)
        coefs.append(cf[idx].astype(np.float32))

    maxc = max(len(i) for i in idxs)
    C = max(256, -(-maxc // 128) * 128)
    nc = _get_nc(C)

    xtT = np.ascontiguousarray(xt.T.astype(np.float16))
    in_maps = []
    for e in range(E):
        n = len(idxs[e])
        xTe = np.zeros((D, C), np.float16)
        xTe[:, :n] = xtT[:, idxs[e]]
        cfull = np.zeros(C, np.float32)
        cfull[:n] = coefs[e]
        coef2d = np.ascontiguousarray(cfull.reshape(C // P, P).T)
        w13 = np.empty((D, HT, 2 * P), np.float16)
        w13[:, :, :P] = w1[e].reshape(D, HT, P)
        w13[:, :, P:] = w3[e].reshape(D, HT, P)
        in_maps.append({
            "xT": xTe,
            "w13": w13,
            "w2": np.ascontiguousarray(w2[e].astype(np.float16)),
            "coef": coef2d,
        })

    res = run_bass_kernel_spmd(nc, in_maps, core_ids=list(range(NCORES)))

    out = np.zeros((T, D), np.float32)
    for e in range(E):
        n = len(idxs[e])
        out[idxs[e]] += res.results[e]["y"][:n]
    return out.reshape(b, s, d)

